# revision 25
# baseline (speedup 1.0000x reference)
"""GATv2 (3-layer, 4-head) on 8 Trainium2 NeuronCores — Bass/Tile SPMD kernel.

v2: all-f16 edge pipeline, dst-block-balanced node permutation, no xr
gather, host-uploaded indicator matrices, chunked AllGather.

Sharding: destination-node partition. Core c owns dst nodes (a balanced
permutation of the original [c*NPC, (c+1)*NPC)) in NBLK blocks of BLK.
Edges (incl. mean-filled self-loops) are bucketed by dst block with
per-core greedy balancing so the max block edge count (=> tpb) is near
the mean. All cores run one shared SPMD program.

Per layer:
  1. node matmuls xl = h@Wl, xr = h@Wr in f16 (own NPC nodes); xl rows
     stream to DRAM chunks, xr blocks stay in SBUF ([BLK,HD] + We rows).
  2. 4 chunked AllGathers of the f16 xl table, overlapping node phase.
  3. per dst block: 1 dma_gather of xl[src] rows (f16), per 128-edge
     tile two matmuls assemble z = ee + xr[dst] + xl[src] in PSUM
     (lhsT = [indbT; feaT] host-uploaded, rhs = [xr_blk; We] in SBUF;
     plus eye @ xlg), one ScalarE Lrelu(alpha=0.2) -> f16,
     block-batched DVE att-mul + per-head reduce, ScalarE exp,
     per-(tile,head) msg scaling, indicator-matmul scatter for out and
     denom, then normalize, head-mean, outer leaky_relu, transpose back.
"""
import sys

sys.path.insert(0, "/opt/trn_rl_repo")
from contextlib import ExitStack

import ml_dtypes
import numpy as np
import concourse.bacc as bacc
import concourse.mybir as mybir
import concourse.tile as tile
from concourse.bass_utils import run_bass_kernel_spmd
from concourse.library_config import mlp

f32 = mybir.dt.float32
f16 = mybir.dt.float16
i16 = mybir.dt.int16
ALU = mybir.AluOpType
AF = mybir.ActivationFunctionType
NPH = np.float16

H = 4
D = 128
HD = H * D
F_IN = 128
NEG = 0.2
N_LAYERS = 3
C = 8
TILE = 128
NCHUNK = 4  # AllGather chunks per layer

# full-problem dims (overridable for small-scale sim tests)
DIMS = dict(N=20000, NPC=2500, BLK=125, NBLK=20)

_BUILD_CACHE = {}


# ----------------------------------------------------------------- host prep
def _pack_idxs(il):
    n = len(il)
    a = np.zeros((128, n // 16), np.int16)
    base = il.reshape(n // 16, 16).T
    for g in range(8):
        a[g * 16:(g + 1) * 16] = base
    return a


def _balance_blocks(deg, nblk, blk):
    """Assign len(deg) nodes to nblk bins of exactly blk nodes, greedily
    balancing the per-bin degree sums. Returns node order (bin-major)."""
    order = np.argsort(-deg, kind="stable")
    bins = [[] for _ in range(nblk)]
    sums = np.zeros(nblk)
    counts = np.zeros(nblk, np.int64)
    for v in order:
        open_bins = np.flatnonzero(counts < blk)
        b = open_bins[np.argmin(sums[open_bins])]
        bins[b].append(v)
        sums[b] += deg[v]
        counts[b] += 1
    return np.concatenate([np.array(b, np.int64) for b in bins])


def _build_shards(edge_index, edge_attr, dims=DIMS):
    N, NPC, BLK, NBLK = dims["N"], dims["NPC"], dims["BLK"], dims["NBLK"]
    CH = NPC // NCHUNK  # AllGather chunk rows per core
    src = np.asarray(edge_index[0], np.int64)
    dst = np.asarray(edge_index[1], np.int64)
    ea = np.asarray(edge_attr, np.float32)
    E = len(src)

    ea_sum = np.zeros((N, 2), np.float32)
    np.add.at(ea_sum, dst, ea)
    cnt = np.bincount(dst, minlength=N).astype(np.float32)
    loop_attr = ea_sum / np.maximum(cnt, 1.0)[:, None]

    fsrc = np.concatenate([src, np.arange(N, dtype=np.int64)])
    fdst = np.concatenate([dst, np.arange(N, dtype=np.int64)])
    ffea = np.concatenate([ea, loop_attr], axis=0)

    # per-core balanced node permutation (new local id = bin-major order)
    deg = np.bincount(fdst, minlength=N).astype(np.int64)
    new_local = np.empty(N, np.int64)   # orig global -> new local id
    orders = []                         # per core: new local -> orig global
    for c in range(C):
        g0 = c * NPC
        order_c = _balance_blocks(deg[g0:g0 + NPC], NBLK, BLK) + g0
        orders.append(order_c)
        new_local[order_c] = np.arange(NPC)

    # src -> row in the chunk-major AllGather table
    src_core = fsrc // NPC
    src_new = new_local[fsrc]
    gather_pos = ((src_new // CH) * (CH * C) + src_core * CH
                  + (src_new % CH))

    # bucket edges by (dst core, dst block)
    dst_core = fdst // NPC
    dst_new = new_local[fdst]
    key = dst_core * NBLK + dst_new // BLK
    eorder = np.argsort(key, kind="stable")
    kb = key[eorder]
    bounds = np.searchsorted(kb, np.arange(C * NBLK + 1))
    max_edges = int(np.max(np.diff(bounds)))
    tpb = (max_edges + TILE - 1) // TILE
    epb = tpb * TILE
    ec = NBLK * epb

    shards = []
    for c in range(C):
        s_src = np.zeros(ec, np.int64)
        s_dstrel = np.full(ec, -1, np.int64)
        s_fea = np.zeros((ec, 2), np.float32)
        for b in range(NBLK):
            k = c * NBLK + b
            el = eorder[bounds[k]:bounds[k + 1]]
            o = b * epb
            n = len(el)
            s_src[o:o + n] = gather_pos[el]
            s_dstrel[o:o + n] = dst_new[el] % BLK
            s_fea[o:o + n] = ffea[el]
        # dstrel cols for on-device indb is_eq: [128, NBLK*tpb]
        rel = s_dstrel.reshape(NBLK, tpb, TILE)
        dstrel = np.ascontiguousarray(
            rel.reshape(NBLK * tpb, TILE).T).astype(np.float32)
        shards.append(dict(
            src_pk=_pack_idxs(s_src.astype(np.int16)),
            dstrel=dstrel,
            feaT=np.ascontiguousarray(s_fea.T).astype(NPH),
            order=orders[c],
        ))
    return shards, tpb


# --------------------------------------------------------------- device build
def _build(tpb, nzb, dims=DIMS, compile=True):
    key = (tpb, nzb, tuple(sorted(dims.items())))
    if key in _BUILD_CACHE:
        return _BUILD_CACHE[key]
    N, NPC, BLK, NBLK = dims["N"], dims["NPC"], dims["BLK"], dims["NBLK"]
    CH = NPC // NCHUNK
    nz_bf, nz_bl, nz_br, nz_bo = nzb
    epb = tpb * TILE
    ec = NBLK * epb
    BPC = NBLK // NCHUNK  # node blocks per AllGather chunk

    nc = bacc.Bacc("TRN2", target_bir_lowering=False, debug=False, num_devices=C)
    d_xT = nc.dram_tensor("xT", [F_IN, NPC], f16, kind="ExternalInput")
    d_srcpk = nc.dram_tensor("src_pk", [128, ec // 16], i16, kind="ExternalInput")
    d_dstrel = nc.dram_tensor("dstrel", [128, NBLK * tpb], f32,
                              kind="ExternalInput")
    d_iorow = nc.dram_tensor("iorow", [128, BLK], f32, kind="ExternalInput")
    d_feaT = nc.dram_tensor("feaT", [2, NBLK * tpb * TILE], f16,
                            kind="ExternalInput")
    d_eye = nc.dram_tensor("eye", [128, 128], f16, kind="ExternalInput")
    d_Wf = nc.dram_tensor("Wf", [F_IN, D], f16, kind="ExternalInput")
    d_Wl = nc.dram_tensor("Wl", [D, HD], f16, kind="ExternalInput")
    d_Wr = nc.dram_tensor("Wr", [D, HD], f16, kind="ExternalInput")
    d_We = nc.dram_tensor("We", [2, HD], f16, kind="ExternalInput")
    d_attb = nc.dram_tensor("att_rep", [128, tpb * HD], f16,
                            kind="ExternalInput")
    d_bf = nc.dram_tensor("bf_col", [128, 1], f32, kind="ExternalInput")
    d_blb = nc.dram_tensor("bl_b", [128, HD], f32, kind="ExternalInput")
    d_brb = nc.dram_tensor("br_b", [128, HD], f32, kind="ExternalInput")
    d_bob = nc.dram_tensor("bo_b", [128, D], f32, kind="ExternalInput")
    d_out = nc.dram_tensor("hout", [NPC, D], f32, kind="ExternalOutput")

    with tile.TileContext(nc) as tc, ExitStack() as ex:
        cst = ex.enter_context(tc.tile_pool(name="cst", bufs=1))
        xrwe = ex.enter_context(tc.tile_pool(name="xrwe", bufs=1))
        dram = ex.enter_context(tc.tile_pool(name="dram", bufs=1, space="DRAM"))
        psZ = ex.enter_context(tc.tile_pool(name="psZ", bufs=4, space="PSUM"))
        psO = ex.enter_context(tc.tile_pool(name="psO", bufs=2, space="PSUM"))
        psD = ex.enter_context(tc.tile_pool(name="psD", bufs=2, space="PSUM"))
        indp = ex.enter_context(tc.tile_pool(name="indp", bufs=2))
        gbuf = ex.enter_context(tc.tile_pool(name="gbuf", bufs=4))
        lzp = ex.enter_context(tc.tile_pool(name="lzp", bufs=2))
        yp = ex.enter_context(tc.tile_pool(name="yp", bufs=2))
        msgp = ex.enter_context(tc.tile_pool(name="msgp", bufs=2))
        blkp = ex.enter_context(tc.tile_pool(name="blkp", bufs=3))
        evp = ex.enter_context(tc.tile_pool(name="evp", bufs=3))

        nc.gpsimd.load_library(mlp)

        def ld(dt, shape, dtype=f16):
            t = cst.tile(shape, dtype, name=f"sb_{dt.name}")
            nc.sync.dma_start(t[:], dt[:])
            return t

        eye = ld(d_eye, [128, 128])
        iorow = ld(d_iorow, [128, BLK], f32)
        dstrel = ld(d_dstrel, [128, NBLK * tpb], f32)
        Wf = ld(d_Wf, [F_IN, D])
        Wl = ld(d_Wl, [D, HD])
        Wr = ld(d_Wr, [D, HD])
        We = ld(d_We, [2, HD])
        attb = ld(d_attb, [128, tpb * HD])
        xT = ld(d_xT, [F_IN, NPC])
        srcpk = ld(d_srcpk, [128, ec // 16], i16)
        bfc = ld(d_bf, [128, 1], f32) if nz_bf else None
        blb = ld(d_blb, [128, HD], f32) if nz_bl else None
        brb = ld(d_brb, [128, HD], f32) if nz_br else None
        bob = ld(d_bob, [128, D], f32) if nz_bo else None

        hT_t = [cst.tile([128, BLK], f16, name=f"hT{m}")
                for m in range(NBLK)]
        # xr blocks (+We rows) stay in SBUF; rewritten each layer
        xrwe_t = [xrwe.tile([BLK + 2, HD], f16, name=f"xrwe{m}")
                  for m in range(NBLK)]
        for m in range(NBLK):
            nc.sync.dma_start(xrwe_t[m][BLK:BLK + 2, :], d_We[:])

        agins = [dram.tile([NPC, HD], f16, name=f"agin{L}")
                 for L in range(N_LAYERS)]
        agouts = [dram.tile([N, HD], f16, name=f"agout{L}")
                  for L in range(N_LAYERS)]

        def node_xl(L, m):
            """xl = h@Wl for block m of layer L -> agin[L] rows."""
            psl = psZ.tile([BLK, HD], f32, tag="psZ")
            nc.tensor.matmul(psl[:], hT_t[m][:], Wl[:], start=True, stop=True)
            xle = evp.tile([BLK, HD], f16, tag="ev")
            if nz_bl:
                nc.vector.tensor_add(xle[:], psl[:], blb[:BLK, :])
            else:
                nc.scalar.activation(xle[:], psl[:], AF.Copy)
            nc.sync.dma_start(agins[L][m * BLK:(m + 1) * BLK, :], xle[:])

        def node_xr(L, m):
            psr = psZ.tile([BLK, HD], f32, tag="psZ")
            nc.tensor.matmul(psr[:], hT_t[m][:], Wr[:], start=True, stop=True)
            if nz_br:
                nc.vector.tensor_add(xrwe_t[m][:BLK, :], psr[:], brb[:BLK, :])
            else:
                nc.scalar.activation(xrwe_t[m][:BLK, :], psr[:], AF.Copy)

        def ag_chunk(L, k):
            nc.gpsimd.collective_compute(
                "AllGather", ALU.bypass,
                replica_groups=[list(range(C))],
                ins=[agins[L][k * CH:(k + 1) * CH, :].opt()],
                outs=[agouts[L][k * CH * C:(k + 1) * CH * C, :].opt()],
            )

        # ---- layer-0 features: h0T_m = Wf.T @ xT_m, then layer-0 node phase
        for m in range(NBLK):
            ps = psZ.tile([128, BLK], f32, tag="psZ")
            nc.tensor.matmul(ps[:], Wf[:], xT[:, m * BLK:(m + 1) * BLK],
                             start=True, stop=True)
            if nz_bf:
                nc.vector.tensor_scalar_add(hT_t[m][:], ps[:], bfc[:])
            else:
                nc.scalar.activation(hT_t[m][:], ps[:], AF.Copy)
        for m in range(NBLK):
            node_xl(0, m)
            if m % BPC == BPC - 1:
                ag_chunk(0, m // BPC)
        for m in range(NBLK):
            node_xr(0, m)

        for L in range(N_LAYERS):
            agout = agouts[L]
            # ---- edge phase, per dst block (next layer's node work inlined)
            for b in range(NBLK):
                e0 = b * epb
                xlg = gbuf.tile([128, tpb, HD], f16, tag="xlg")
                for g0 in range(0, tpb, 4):
                    g1 = min(g0 + 4, tpb)
                    ne = (g1 - g0) * TILE
                    c0 = (e0 + g0 * TILE) // 16
                    nc.gpsimd.dma_gather(xlg[:, g0:g1, :], agout[:],
                                         srcpk[:, c0:c0 + ne // 16],
                                         ne, ne, HD)
                indb = indp.tile([128, tpb * BLK], f16, tag="indb")
                for t in range(tpb):
                    nc.vector.tensor_scalar(
                        indb[:, t * BLK:(t + 1) * BLK], iorow[:],
                        dstrel[:, b * tpb + t:b * tpb + t + 1], None,
                        ALU.is_equal)
                # stack tile [indbT; feaT]: transposed indicators + features
                ifea = indp.tile([BLK + 2, epb], f16, tag="ifea")
                nc.sync.dma_start(ifea[BLK:BLK + 2, :],
                                  d_feaT[:, e0:e0 + epb])
                for t in range(tpb):
                    tpp = psZ.tile([BLK, TILE], f16, tag="psZ")
                    nc.tensor.transpose(tpp[:],
                                        indb[:, t * BLK:(t + 1) * BLK], eye[:])
                    nc.scalar.activation(
                        ifea[:BLK, t * TILE:(t + 1) * TILE], tpp[:], AF.Copy)
                lz = lzp.tile([128, tpb, HD], f16, tag="lz")
                for t in range(tpb):
                    zp = psZ.tile([128, HD], f32, tag="psZ")
                    nc.tensor.matmul(zp[:], ifea[:, t * TILE:(t + 1) * TILE],
                                     xrwe_t[b][:], start=True, stop=False)
                    nc.tensor.matmul(zp[:], eye[:], xlg[:, t, :],
                                     start=False, stop=True)
                    nc.scalar.activation(lz[:, t, :], zp[:], AF.Prelu,
                                         alpha=NEG)
                y = yp.tile([128, tpb, HD], f16, tag="y")
                nc.vector.tensor_mul(
                    y[:].rearrange("p a b -> p (a b)"),
                    lz[:].rearrange("p a b -> p (a b)"), attb[:])
                y4 = y[:].rearrange("p t (h u d) -> p t h u d", h=H, u=2)
                r1 = blkp.tile([128, tpb, H, D // 2], f16, tag="r1")
                nc.vector.tensor_add(r1[:], y4[:, :, :, 0, :], y4[:, :, :, 1, :])
                r2t = r1[:].rearrange("p t h (u d) -> p t h u d", u=2)
                r2 = blkp.tile([128, tpb, H, D // 4], f16, tag="r2")
                nc.vector.tensor_add(r2[:], r2t[:, :, :, 0, :], r2t[:, :, :, 1, :])
                lgb = blkp.tile([128, tpb, H], f32, tag="lgb")
                nc.vector.tensor_reduce(
                    lgb[:], r2[:],
                    axis=mybir.AxisListType.X, op=ALU.add)
                msg = msgp.tile([128, tpb, HD + H], f16, tag="msg")
                web = blkp.tile([128, tpb, H], f32, tag="web")
                nc.scalar.activation(web[:], lgb[:], AF.Exp)
                nc.vector.tensor_copy(msg[:, :, HD:], web[:])
                for t in range(tpb):
                    for hh in range(H):
                        nc.vector.tensor_scalar_mul(
                            msg[:, t, hh * D:(hh + 1) * D],
                            xlg[:, t, hh * D:(hh + 1) * D],
                            web[:, t, hh:hh + 1])
                outp = psO.tile([BLK, HD], f32, tag="psO")
                denp = psD.tile([BLK, H], f32, tag="psD")
                for t in range(tpb):
                    nc.tensor.matmul(outp[:], indb[:, t * BLK:(t + 1) * BLK],
                                     msg[:, t, :HD],
                                     start=(t == 0), stop=(t == tpb - 1))
                    nc.tensor.matmul(denp[:], indb[:, t * BLK:(t + 1) * BLK],
                                     msg[:, t, HD:],
                                     start=(t == 0), stop=(t == tpb - 1))
                invd = blkp.tile([BLK, H], f32, tag="invd")
                nc.vector.reciprocal(invd[:], denp[:])
                # fold the head-mean 1/H into the normalizer
                nc.vector.tensor_scalar_mul(invd[:], invd[:], 1.0 / H)
                o = blkp.tile([BLK, H, D], f16, tag="o")
                for hh in range(H):
                    nc.scalar.activation(o[:, hh, :],
                                         outp[:, hh * D:(hh + 1) * D],
                                         AF.Copy, scale=invd[:, hh:hh + 1])
                s01 = blkp.tile([BLK, D], f32, tag="s01")
                nc.vector.tensor_add(s01[:], o[:, 0, :], o[:, 1, :])
                s23 = blkp.tile([BLK, D], f32, tag="s23")
                nc.vector.tensor_add(s23[:], o[:, 2, :], o[:, 3, :])
                sm = blkp.tile([BLK, D], f32, tag="sm")
                nc.vector.tensor_add(sm[:], s01[:], s23[:])
                if nz_bo:
                    nc.vector.tensor_add(sm[:], sm[:], bob[:BLK, :])
                if L == N_LAYERS - 1:
                    hb = blkp.tile([BLK, D], f32, tag="hb32")
                    nc.vector.scalar_tensor_tensor(
                        hb[:], sm[:], 0.01, sm[:], ALU.mult, ALU.max)
                    nc.sync.dma_start(d_out[b * BLK:(b + 1) * BLK, :], hb[:])
                else:
                    hb = blkp.tile([BLK, D], f16, tag="hb")
                    nc.vector.scalar_tensor_tensor(
                        hb[:], sm[:], 0.01, sm[:], ALU.mult, ALU.max)
                    tp = psZ.tile([128, BLK], f16, tag="psZ")
                    nc.tensor.transpose(tp[:], hb[:], eye[:BLK, :BLK])
                    nc.scalar.activation(hT_t[b][:], tp[:], AF.Copy)
                    # next layer's node work for this block, now that
                    # hT_t[b] is fresh and xrwe_t[b] is no longer read
                    node_xl(L + 1, b)
                    node_xr(L + 1, b)
                    if b % BPC == BPC - 1:
                        ag_chunk(L + 1, b // BPC)

    if compile:
        nc.compile()
    _BUILD_CACHE[key] = nc
    return nc


# ------------------------------------------------------------------ in_maps
def make_in_maps(inputs, dims=DIMS):
    N, NPC, BLK = dims["N"], dims["NPC"], dims["BLK"]
    x = np.asarray(inputs["x"], np.float32)
    Wf = np.asarray(inputs["Wf"], np.float32)
    bf = np.asarray(inputs["bf"], np.float32)
    Wl = np.asarray(inputs["Wl"], np.float32)
    bl = np.asarray(inputs["bl"], np.float32)
    Wr = np.asarray(inputs["Wr"], np.float32)
    br = np.asarray(inputs["br"], np.float32)
    We = np.asarray(inputs["We"], np.float32)
    att = np.asarray(inputs["att"], np.float32)
    bias_out = np.asarray(inputs["bias_out"], np.float32)

    global _LAST_ORDERS
    shards, tpb = _build_shards(inputs["edge_index"], inputs["edge_attr"], dims)
    _LAST_ORDERS = [sh["order"] for sh in shards]
    nzb = (bool(bf.any()), bool(bl.any()), bool(br.any()), bool(bias_out.any()))

    common = dict(
        eye=np.eye(128, dtype=np.float32).astype(NPH),
        iorow=np.tile(np.arange(BLK, dtype=np.float32), (128, 1)),
        Wf=np.ascontiguousarray(Wf).astype(NPH),
        Wl=np.ascontiguousarray(Wl).astype(NPH),
        Wr=np.ascontiguousarray(Wr).astype(NPH),
        We=np.ascontiguousarray(We).astype(NPH),
        att_rep=np.tile(att.reshape(1, HD), (128, tpb)).astype(NPH),
        bf_col=np.ascontiguousarray(bf.reshape(D, 1)),
        bl_b=np.tile(bl.reshape(1, HD), (128, 1)).astype(np.float32),
        br_b=np.tile(br.reshape(1, HD), (128, 1)).astype(np.float32),
        bo_b=np.tile(bias_out.reshape(1, D), (128, 1)).astype(np.float32),
    )
    in_maps = []
    for c in range(C):
        sh = shards[c]
        m = dict(common)
        m["xT"] = np.ascontiguousarray(x[sh["order"]].T).astype(NPH)
        m["src_pk"] = sh["src_pk"]
        m["dstrel"] = sh["dstrel"]
        m["feaT"] = sh["feaT"]
        in_maps.append(m)
    return in_maps, tpb, nzb


# -------------------------------------------------------------------- kernel
_LAST_ORDERS = None


def assemble(res, dims=DIMS):
    """Un-permute per-core outputs back to original node order."""
    N = dims["N"]
    out = np.empty((N, D), np.float32)
    for c in range(C):
        out[_LAST_ORDERS[c]] = res.results[c]["hout"]
    return out


def kernel(**inputs):
    in_maps, tpb, nzb = make_in_maps(inputs, DIMS)
    nc = _build(tpb, nzb, DIMS)
    res = run_bass_kernel_spmd(nc, in_maps, list(range(C)))
    return assemble(res, DIMS)


if __name__ == "__main__":
    nc = _build(9, (False, False, False, False), DIMS, compile=False)
    print("trace-only build OK")
